# revision 11
# baseline (speedup 1.0000x reference)
"""Trainium2 Bass kernel for the MessagePassingGNN problem.

Full inputs in, full outputs out. Internally shards the pairwise edge
computation along the i axis across 8 NeuronCores (each core computes
edge rows for its 32-i slab plus the node head for the same slab).

Structure (per core):
  aggT = sum_n xT / N                  (DVE free-dim reduce)
  upd  = MLP(agg)                      (tiny bf16 matmuls, free dim 2)
  uT   = xT + upd[b]                   (per-partition-bias adds)
  hjT  = (u @ et_w1[D:])T              (bf16 matmuls, d on partitions)
  hib  = (u_slab @ et_w1[:D])T + et_b1
  node head on the slab
  edge: for each (i, d-chunk):  R = relu(hjT + hib[:,i])  [128, 512] bf16
        (fused add+max on DVE/GPSIMD/ACT), then PE matmul with w2 chunk
        [128,1] reduces over d into one PSUM row per i-pair; sigmoid+DMA.
The [B,N,N,D] hidden tensor of the reference is never materialized.
"""

import numpy as np
import ml_dtypes

import concourse.bass as bass
import concourse.bacc as bacc
import concourse.mybir as mybir
import concourse.tile as tile
from concourse.alu_op_type import AluOpType
from concourse.bass_utils import run_bass_kernel_spmd

F32 = mybir.dt.float32
BF16 = mybir.dt.bfloat16
AF = mybir.ActivationFunctionType

B, N, D = 2, 256, 512
NCORES = 8
ISLAB = N // NCORES          # 32 i-rows per core
R = B * N                    # 512 flattened rows (b, n)
RS = B * ISLAB               # 64 flattened slab rows (b, ii)

# relu half-op engine assignment pattern (cycled):
#   V = vector/DVE, G = gpsimd (do not use: ~14x slower + port contention),
#   A = scalar/ACT
HALF_PATTERN = "VVA"

_CACHE = {}


def _build():
    nc = bacc.Bacc("TRN2", target_bir_lowering=False, debug=False,
                   num_devices=NCORES)

    # ---------------- DRAM I/O ----------------
    d_xT = nc.dram_tensor("xT", [D, R], F32, kind="ExternalInput")
    d_xsT = nc.dram_tensor("xsT", [D, RS], F32, kind="ExternalInput")
    d_wnu1 = nc.dram_tensor("wnu1", [D, D], BF16, kind="ExternalInput")
    d_bnu1 = nc.dram_tensor("bnu1", [D], F32, kind="ExternalInput")
    d_wnu2 = nc.dram_tensor("wnu2", [D, D], BF16, kind="ExternalInput")
    d_bnu2 = nc.dram_tensor("bnu2", [D], F32, kind="ExternalInput")
    d_wnt1 = nc.dram_tensor("wnt1", [D, D], BF16, kind="ExternalInput")
    d_bnt1 = nc.dram_tensor("bnt1", [D], F32, kind="ExternalInput")
    d_wnt2 = nc.dram_tensor("wnt2", [D, 3], BF16, kind="ExternalInput")
    d_bnt2 = nc.dram_tensor("bnt2", [1, 3], BF16, kind="ExternalInput")
    d_wea = nc.dram_tensor("wea", [D, D], BF16, kind="ExternalInput")
    d_web = nc.dram_tensor("web", [D, D], BF16, kind="ExternalInput")
    d_be1 = nc.dram_tensor("be1", [D], F32, kind="ExternalInput")
    d_we2 = nc.dram_tensor("we2", [D], F32, kind="ExternalInput")
    d_be2 = nc.dram_tensor("be2", [1, 1], BF16, kind="ExternalInput")

    d_node = nc.dram_tensor("node_o", [RS, 3], F32, kind="ExternalOutput")
    d_edge = nc.dram_tensor("edge_o", [RS, N], F32, kind="ExternalOutput")

    with tile.TileContext(nc) as tc:
        with tc.tile_pool(name="const", bufs=1) as cp:
            # ---------------- SBUF residents ----------------
            t_xT = cp.tile([128, 4 * R], F32, tag="xT")        # [p, q*512+r]
            t_xsT = cp.tile([128, 4 * RS], F32, tag="xsT")     # [p, q*64+rs]
            t_uT = cp.tile([128, 4 * R], F32, tag="uT")
            t_uTb = cp.tile([128, 4 * R], BF16, tag="uTb")
            t_usT = cp.tile([128, 4 * RS], F32, tag="usT")
            t_usTb = cp.tile([128, 4 * RS], BF16, tag="usTb")
            t_wnu1 = cp.tile([128, 4 * D], BF16, tag="wnu1")   # [p, kq*512+o]
            t_wnu2 = cp.tile([128, 4 * D], BF16, tag="wnu2")
            t_wnt1 = cp.tile([128, 4 * D], BF16, tag="wnt1")
            t_wea = cp.tile([128, 4 * D], BF16, tag="wea")
            t_web = cp.tile([128, 4 * D], BF16, tag="web")
            t_wnt2 = cp.tile([128, 12], BF16, tag="wnt2")      # [p, kq*3+t]
            t_bnu1 = cp.tile([128, 4], F32, tag="bnu1")        # [p, chunk]
            t_bnu2 = cp.tile([128, 4], F32, tag="bnu2")
            t_bnt1 = cp.tile([128, 4], F32, tag="bnt1")
            t_be1 = cp.tile([128, 4], F32, tag="be1")
            t_we2 = cp.tile([128, 4], F32, tag="we2")
            t_we2b = cp.tile([128, 4], BF16, tag="we2b")
            t_bnt2 = cp.tile([1, 3], BF16, tag="bnt2")
            t_be2 = cp.tile([1, 1], BF16, tag="be2")
            t_ones = cp.tile([1, 128], BF16, tag="ones")
            t_dummy = cp.tile([1, 1], F32, tag="dummy")

            t_aggs = cp.tile([128, 8], F32, tag="aggs")        # [p, q*2+b]
            t_aggb = cp.tile([128, 8], BF16, tag="aggb")
            t_h1b = cp.tile([128, 8], BF16, tag="h1b")
            t_upd = cp.tile([128, 8], F32, tag="upd")
            t_b2col = cp.tile([128, 1], F32, tag="b2col")      # be2 bcast

            t_hjb = cp.tile([128, 4 * R], BF16, tag="hjb")     # [p, q*512+(b*256+j)]
            t_hib = cp.tile([128, 4 * RS], F32, tag="hib")     # [p, q*64+(b*32+ii)]
            t_h1n = cp.tile([128, 4 * RS], BF16, tag="h1n")
            t_nout = cp.tile([RS, 3], F32, tag="nout")

            # ------- input DMAs (split across the three DMA rings so the
            # critical-path loads don't serialize behind the rest) -------
            sdma = nc.sync.dma_start        # HWDGE ring 1: critical path
            adma = nc.scalar.dma_start      # HWDGE ring 2
            gdma = nc.gpsimd.dma_start      # SWDGE: non-critical
            sdma(t_xT[:, :], d_xT.ap().rearrange("(q p) r -> p q r", p=128))
            adma(t_xsT[:, :], d_xsT.ap().rearrange("(q p) s -> p q s", p=128))
            sdma(t_wnu1[:, :], d_wnu1.ap().rearrange("(k p) o -> p k o", p=128))
            adma(t_wnu2[:, :], d_wnu2.ap().rearrange("(k p) o -> p k o", p=128))
            sdma(t_web[:, :], d_web.ap().rearrange("(k p) o -> p k o", p=128))
            adma(t_wea[:, :], d_wea.ap().rearrange("(k p) o -> p k o", p=128))
            adma(t_we2[:, :], d_we2.ap().rearrange("(c p) -> p c", p=128))
            adma(t_be1[:, :], d_be1.ap().rearrange("(c p) -> p c", p=128))
            gdma(t_be2[:, :], d_be2.ap())
            adma(t_bnu1[:, :], d_bnu1.ap().rearrange("(c p) -> p c", p=128))
            adma(t_bnu2[:, :], d_bnu2.ap().rearrange("(c p) -> p c", p=128))
            gdma(t_bnt1[:, :], d_bnt1.ap().rearrange("(c p) -> p c", p=128))
            gdma(t_bnt2[:, :], d_bnt2.ap())
            gdma(t_wnt1[:, :], d_wnt1.ap().rearrange("(k p) o -> p k o", p=128))
            gdma(t_wnt2[:, :], d_wnt2.ap().rearrange("(k p) t -> p k t", p=128))

            nc.gpsimd.memset(t_ones[:, :], 1.0)
            # trigger the sigmoid activation-table load as early as possible
            nc.scalar.activation(t_dummy[:, :], t_we2[0:1, 0:1], AF.Sigmoid)

            nc.vector.tensor_copy(t_we2b[:, :], t_we2[:, :])

            # agg = sum_n x (per b, per d-chunk), scaled to mean
            for q in range(4):
                for b in range(B):
                    nc.vector.tensor_reduce(
                        t_aggs[:, q * 2 + b: q * 2 + b + 1],
                        t_xT[:, q * R + b * N: q * R + (b + 1) * N],
                        mybir.AxisListType.X, AluOpType.add)
            nc.vector.tensor_scalar(t_aggb[:, :], t_aggs[:, :], 1.0 / N, None,
                                    AluOpType.mult)

            with (
                tc.tile_pool(name="ps_mlp", bufs=2, space="PSUM") as pmlp,
                tc.tile_pool(name="ps_hjx", bufs=2, space="PSUM") as phjx,
                tc.tile_pool(name="ps_misc", bufs=1, space="PSUM") as pmisc,
            ):
                # ---- node-update MLP on agg (tiny: free dim = 2) ----
                for oq in range(4):
                    ps = pmlp.tile([128, 2], F32, tag="mlp")
                    for kq in range(4):
                        nc.tensor.matmul(
                            ps[:, :],
                            t_wnu1[:, kq * D + oq * 128: kq * D + (oq + 1) * 128],
                            t_aggb[:, kq * 2: (kq + 1) * 2],
                            start=(kq == 0), stop=(kq == 3))
                    nc.vector.tensor_scalar(
                        t_h1b[:, oq * 2: (oq + 1) * 2], ps[:, :],
                        t_bnu1[:, oq: oq + 1], 0.0,
                        AluOpType.add, AluOpType.max)
                for oq in range(4):
                    ps = pmlp.tile([128, 2], F32, tag="mlp")
                    for kq in range(4):
                        nc.tensor.matmul(
                            ps[:, :],
                            t_wnu2[:, kq * D + oq * 128: kq * D + (oq + 1) * 128],
                            t_h1b[:, kq * 2: (kq + 1) * 2],
                            start=(kq == 0), stop=(kq == 3))
                    nc.vector.tensor_scalar(
                        t_upd[:, oq * 2: (oq + 1) * 2], ps[:, :],
                        t_bnu2[:, oq: oq + 1], None, AluOpType.add)

                # ---- uT = xT + upd[b];  usT = xsT + upd[b] ----
                for q in range(4):
                    for b in range(B):
                        dst = t_uT[:, q * R + b * N: q * R + (b + 1) * N]
                        src = t_xT[:, q * R + b * N: q * R + (b + 1) * N]
                        sc = t_upd[:, q * 2 + b: q * 2 + b + 1]
                        if b == 0:
                            nc.vector.tensor_scalar(dst, src, sc, None, AluOpType.add)
                        else:
                            nc.scalar.activation(dst, src, AF.Identity, bias=sc)
                        dsts = t_usT[:, q * RS + b * ISLAB: q * RS + (b + 1) * ISLAB]
                        srcs = t_xsT[:, q * RS + b * ISLAB: q * RS + (b + 1) * ISLAB]
                        nc.vector.tensor_scalar(dsts, srcs, sc, None, AluOpType.add)
                # big casts on ACT (idle during this window)
                nc.scalar.activation(t_uTb[:, :], t_uT[:, :], AF.Identity)
                nc.scalar.activation(t_usTb[:, :], t_usT[:, :], AF.Identity)

                # ---- be2 broadcast across partitions (for edge sigmoid bias)
                psb = pmisc.tile([128, 1], F32, tag="bb")
                nc.tensor.matmul(psb[:, :], t_ones[0:1, 0:128], t_be2[0:1, 0:1],
                                 start=True, stop=True)
                nc.vector.tensor_copy(t_b2col[:, :], psb[:, :])

                # ---- hjT = (u@web)T  (bf16, full j range) ----
                for oq in range(4):
                    psj = phjx.tile([128, R], F32, tag="hjx")
                    for kq in range(4):
                        nc.tensor.matmul(
                            psj[:, :],
                            t_web[:, kq * D + oq * 128: kq * D + (oq + 1) * 128],
                            t_uTb[:, kq * R: (kq + 1) * R],
                            start=(kq == 0), stop=(kq == 3))
                    nc.vector.tensor_copy(t_hjb[:, oq * R: (oq + 1) * R], psj[:, :])

                # ---- hiT slab = (us@wea)T + be1  (fp32) ----
                psh = pmisc.tile([128, 4 * RS], F32, tag="hi")
                for oq in range(4):
                    for kq in range(4):
                        nc.tensor.matmul(
                            psh[:, oq * RS: (oq + 1) * RS],
                            t_wea[:, kq * D + oq * 128: kq * D + (oq + 1) * 128],
                            t_usTb[:, kq * RS: (kq + 1) * RS],
                            start=(kq == 0), stop=(kq == 3))
                for oq in range(4):
                    nc.vector.tensor_scalar(
                        t_hib[:, oq * RS: (oq + 1) * RS],
                        psh[:, oq * RS: (oq + 1) * RS],
                        t_be1[:, oq: oq + 1], None, AluOpType.add)

            # ---- node head (completes before the edge phase needs PSUM) ----
            with tc.tile_pool(name="ps_node", bufs=1, space="PSUM") as pnode:
                psn = pnode.tile([128, 4 * RS], F32, tag="n1")
                for oq in range(4):
                    for kq in range(4):
                        nc.tensor.matmul(
                            psn[:, oq * RS: (oq + 1) * RS],
                            t_wnt1[:, kq * D + oq * 128: kq * D + (oq + 1) * 128],
                            t_usTb[:, kq * RS: (kq + 1) * RS],
                            start=(kq == 0), stop=(kq == 3))
                for oq in range(4):
                    nc.vector.tensor_scalar(
                        t_h1n[:, oq * RS: (oq + 1) * RS],
                        psn[:, oq * RS: (oq + 1) * RS],
                        t_bnt1[:, oq: oq + 1], 0.0,
                        AluOpType.add, AluOpType.max)
                psn2 = pnode.tile([RS, 3], F32, tag="n2")
                for kq in range(4):
                    nc.tensor.matmul(
                        psn2[:, :],
                        t_h1n[:, kq * RS: (kq + 1) * RS],
                        t_wnt2[:, kq * 3: (kq + 1) * 3],
                        start=(kq == 0), stop=False)
                nc.tensor.matmul(psn2[:, :], t_ones[0:1, 0:RS], t_bnt2[0:1, :],
                                 start=False, stop=True)
                nc.scalar.activation(t_nout[:, :], psn2[:, :], AF.Sigmoid)
                nc.sync.dma_start(d_node.ap()[:, :], t_nout[:, :])

            # ---- edge main loop ----
            with (
                tc.tile_pool(name="ps_edge", bufs=1, space="PSUM") as pedge,
                tc.tile_pool(name="rpool", bufs=16) as rp,
                tc.tile_pool(name="stage", bufs=2) as stp,
            ):
                pes = []
                for b in range(B):
                    pe_t = pedge.tile([128, 4 * D], F32, tag=f"pe{b}")
                    pes.append(pe_t)
                half_ctr = 0
                for oq in range(4):
                    for b in range(B):
                        for s in range(4):
                            for m in range(4):
                                rt = rp.tile([128, D], BF16, tag="rt")
                                for h in range(2):
                                    i = 8 * s + 2 * m + h
                                    dst = rt[:, h * N: (h + 1) * N]
                                    src = t_hjb[:, oq * R + b * N: oq * R + (b + 1) * N]
                                    sc = t_hib[:, oq * RS + b * ISLAB + i:
                                               oq * RS + b * ISLAB + i + 1]
                                    e = HALF_PATTERN[half_ctr % len(HALF_PATTERN)]
                                    half_ctr += 1
                                    if e == "A":
                                        nc.scalar.activation(dst, src, AF.Relu, bias=sc)
                                    elif e == "G":
                                        nc.gpsimd.tensor_scalar(
                                            dst, src, sc, 0.0,
                                            AluOpType.add, AluOpType.max)
                                    else:
                                        nc.vector.tensor_scalar(
                                            dst, src, sc, 0.0,
                                            AluOpType.add, AluOpType.max)
                                nc.tensor.matmul(
                                    pes[b][32 * m: 32 * m + 1, s * D: (s + 1) * D],
                                    t_we2b[:, oq: oq + 1],
                                    rt[:, :],
                                    start=(oq == 0), stop=(oq == 3),
                                    tile_position=(0, 32 * m))
                for b in range(B):
                    stage = stp.tile([128, 4 * D], F32, tag="stage")
                    nc.scalar.activation(stage[:, :], pes[b][:, :], AF.Sigmoid,
                                         bias=t_b2col[:, :])
                    eb = d_edge.ap()[b * ISLAB: (b + 1) * ISLAB, :].rearrange(
                        "(s m h) j -> m s h j", s=4, m=4, h=2)
                    for m in range(4):
                        nc.sync.dma_start(eb[m], stage[32 * m: 32 * m + 1, :])

    nc.finalize()
    return nc


def _get_nc():
    if "nc" not in _CACHE:
        _CACHE["nc"] = _build()
    return _CACHE["nc"]


def _prepare_in_maps(inputs):
    bf = ml_dtypes.bfloat16
    x = np.asarray(inputs["node_features"], dtype=np.float32)
    xT = np.ascontiguousarray(x.reshape(B * N, D).T)          # [D, R]
    et_w1 = np.asarray(inputs["et_w1"], np.float32)
    base = {
        "xT": xT,
        "wnu1": np.asarray(inputs["nu_w1"], np.float32).astype(bf),
        "bnu1": np.asarray(inputs["nu_b1"], np.float32),
        "wnu2": np.asarray(inputs["nu_w2"], np.float32).astype(bf),
        "bnu2": np.asarray(inputs["nu_b2"], np.float32),
        "wnt1": np.asarray(inputs["nt_w1"], np.float32).astype(bf),
        "bnt1": np.asarray(inputs["nt_b1"], np.float32),
        "wnt2": np.asarray(inputs["nt_w2"], np.float32).astype(bf),
        "bnt2": np.asarray(inputs["nt_b2"], np.float32).reshape(1, 3).astype(bf),
        "wea": np.ascontiguousarray(et_w1[:D]).astype(bf),
        "web": np.ascontiguousarray(et_w1[D:]).astype(bf),
        "be1": np.asarray(inputs["et_b1"], np.float32),
        "we2": np.ascontiguousarray(np.asarray(inputs["et_w2"], np.float32)[:, 0]),
        "be2": np.asarray(inputs["et_b2"], np.float32).reshape(1, 1).astype(bf),
    }
    in_maps = []
    for c in range(NCORES):
        xs = x[:, c * ISLAB: (c + 1) * ISLAB, :]              # [B, 32, D]
        xsT = np.ascontiguousarray(xs.reshape(RS, D).T)       # [D, RS]
        in_maps.append({**base, "xsT": xsT})
    return in_maps


def _gather(results):
    node = np.zeros((B, N, 3), np.float32)
    edge = np.zeros((B, N, N), np.float32)
    for c in range(NCORES):
        node[:, c * ISLAB: (c + 1) * ISLAB, :] = \
            results[c]["node_o"].reshape(B, ISLAB, 3)
        edge[:, c * ISLAB: (c + 1) * ISLAB, :] = \
            results[c]["edge_o"].reshape(B, ISLAB, N)
    return node.reshape(B, N * 3), edge.reshape(B, N * N)


def _run(inputs, trace=False, **kwargs):
    nc = _get_nc()
    in_maps = _prepare_in_maps(inputs)
    res = run_bass_kernel_spmd(nc, in_maps, list(range(NCORES)),
                               trace=trace, **kwargs)
    return _gather(res.results), res


def kernel(**inputs):
    (node, edge), _ = _run(inputs, trace=False)
    return node, edge


def kernel_traced(**inputs):
    """Returns ((node, edge), BassKernelResults-with-profile)."""
    return _run(inputs, trace=True)


# revision 18
# speedup vs baseline: 1.1325x; 1.1325x over previous
"""Trainium2 Bass kernel for the MessagePassingGNN problem.

Full inputs in, full outputs out. Internally shards the pairwise edge
computation along the i axis across 8 NeuronCores (each core computes
edge rows for its 32-i slab plus the node head for the same slab).

Structure (per core):
  aggT = sum_n xT / N                  (DVE free-dim reduce)
  upd  = MLP(agg)                      (tiny bf16 matmuls, free dim 2)
  uT   = xT + upd[b]                   (per-partition-bias adds)
  hjT  = (u @ et_w1[D:])T              (bf16 matmuls, d on partitions)
  hib  = (u_slab @ et_w1[:D])T + et_b1
  node head on the slab
  edge: for each (i, d-chunk):  R = relu(hjT + hib[:,i])  [128, 512] bf16
        (fused add+max on DVE/GPSIMD/ACT), then PE matmul with w2 chunk
        [128,1] reduces over d into one PSUM row per i-pair; sigmoid+DMA.
The [B,N,N,D] hidden tensor of the reference is never materialized.
"""

import numpy as np
import ml_dtypes

import concourse.bass as bass
import concourse.bacc as bacc
import concourse.mybir as mybir
import concourse.tile as tile
from concourse.alu_op_type import AluOpType
from concourse.bass_utils import run_bass_kernel_spmd

F32 = mybir.dt.float32
BF16 = mybir.dt.bfloat16
AF = mybir.ActivationFunctionType

B, N, D = 2, 256, 512
NCORES = 8
ISLAB = N // NCORES          # 32 i-rows per core
R = B * N                    # 512 flattened rows (b, n)
RS = B * ISLAB               # 64 flattened slab rows (b, ii)

# relu half-op engine assignment pattern (cycled):
#   V = vector/DVE, G = gpsimd (do not use: ~14x slower + port contention),
#   A = scalar/ACT
HALF_PATTERN = "VVA"

_CACHE = {}


def _build():
    nc = bacc.Bacc("TRN2", target_bir_lowering=False, debug=False,
                   num_devices=NCORES)

    # ---------------- DRAM I/O ----------------
    d_xT = nc.dram_tensor("xT", [D, R], F32, kind="ExternalInput")
    d_xsT = nc.dram_tensor("xsT", [D, RS], F32, kind="ExternalInput")
    d_wnu1 = nc.dram_tensor("wnu1", [D, D], BF16, kind="ExternalInput")
    d_bnu1 = nc.dram_tensor("bnu1", [D], F32, kind="ExternalInput")
    d_wnu2 = nc.dram_tensor("wnu2", [D, D], BF16, kind="ExternalInput")
    d_bnu2 = nc.dram_tensor("bnu2", [D], F32, kind="ExternalInput")
    d_wnt1 = nc.dram_tensor("wnt1", [D, D], BF16, kind="ExternalInput")
    d_bnt1 = nc.dram_tensor("bnt1", [D], F32, kind="ExternalInput")
    d_wnt2 = nc.dram_tensor("wnt2", [D, 3], BF16, kind="ExternalInput")
    d_bnt2 = nc.dram_tensor("bnt2", [1, 3], BF16, kind="ExternalInput")
    d_wea = nc.dram_tensor("wea", [D, D], BF16, kind="ExternalInput")
    d_web = nc.dram_tensor("web", [D, D], BF16, kind="ExternalInput")
    d_be1 = nc.dram_tensor("be1", [D], F32, kind="ExternalInput")
    d_we2 = nc.dram_tensor("we2", [D], F32, kind="ExternalInput")
    d_be2 = nc.dram_tensor("be2", [1, 1], BF16, kind="ExternalInput")

    d_node = nc.dram_tensor("node_o", [RS, 3], F32, kind="ExternalOutput")
    d_edge = nc.dram_tensor("edge_o", [RS, N], F32, kind="ExternalOutput")

    with tile.TileContext(nc) as tc:
        with tc.tile_pool(name="const", bufs=1) as cp:
            # ---------------- SBUF residents ----------------
            t_xT = cp.tile([128, 4 * R], F32, tag="xT")        # [p, q*512+r]
            t_xTb = cp.tile([128, 4 * R], BF16, tag="xTb")
            t_xsT = cp.tile([128, 4 * RS], F32, tag="xsT")     # [p, q*64+rs]
            t_usT = cp.tile([128, 4 * RS], F32, tag="usT")
            t_usTb = cp.tile([128, 4 * RS], BF16, tag="usTb")
            t_wnu1 = cp.tile([128, 4 * D], BF16, tag="wnu1")   # [p, kq*512+o]
            t_wnu2 = cp.tile([128, 4 * D], BF16, tag="wnu2")
            t_wnt1 = cp.tile([128, 4 * D], BF16, tag="wnt1")
            t_wea = cp.tile([128, 4 * D], BF16, tag="wea")
            t_web = cp.tile([128, 4 * D], BF16, tag="web")
            t_wnt2 = cp.tile([128, 12], BF16, tag="wnt2")      # [p, kq*3+t]
            t_bnu1 = cp.tile([128, 4], F32, tag="bnu1")        # [p, chunk]
            t_bnu2 = cp.tile([128, 4], F32, tag="bnu2")
            t_bnt1 = cp.tile([128, 4], F32, tag="bnt1")
            t_be1 = cp.tile([128, 4], F32, tag="be1")
            t_we2 = cp.tile([128, 4], F32, tag="we2")
            t_we2b = cp.tile([128, 4], BF16, tag="we2b")
            t_bnt2 = cp.tile([1, 3], BF16, tag="bnt2")
            t_be2 = cp.tile([1, 1], BF16, tag="be2")
            t_ones = cp.tile([1, 128], BF16, tag="ones")
            t_dummy = cp.tile([1, 1], F32, tag="dummy")

            t_aggs = cp.tile([128, 8], F32, tag="aggs")        # [p, q*2+b]
            t_aggb = cp.tile([128, 8], BF16, tag="aggb")
            t_h1b = cp.tile([128, 8], BF16, tag="h1b")
            t_upd = cp.tile([128, 8], F32, tag="upd")
            t_updb = cp.tile([128, 8], BF16, tag="updb")
            t_biasE = cp.tile([128, 8], F32, tag="biasE")      # upd@web cols
            t_b2col = cp.tile([128, 1], F32, tag="b2col")      # be2 bcast

            t_hjb = cp.tile([128, 4 * R], BF16, tag="hjb")     # [p, q*512+(b*256+j)]
            t_hib = cp.tile([128, 4 * RS], F32, tag="hib")     # [p, q*64+(b*32+ii)]
            t_h1n = cp.tile([128, 4 * RS], BF16, tag="h1n")
            t_nout = cp.tile([RS, 3], F32, tag="nout")

            # ------- input DMAs (split across the three DMA rings so the
            # critical-path loads don't serialize behind the rest) -------
            sdma = nc.sync.dma_start        # HWDGE: critical path loads
            gdma = nc.gpsimd.dma_start      # SWDGE ring: the rest
            sdma(t_xT[:, :], d_xT.ap().rearrange("(q p) r -> p q r", p=128))
            sdma(t_web[:, :], d_web.ap().rearrange("(k p) o -> p k o", p=128))
            sdma(t_wnu1[:, :], d_wnu1.ap().rearrange("(k p) o -> p k o", p=128))
            sdma(t_wnu2[:, :], d_wnu2.ap().rearrange("(k p) o -> p k o", p=128))
            gdma(t_xsT[:, :], d_xsT.ap().rearrange("(q p) s -> p q s", p=128))
            gdma(t_wea[:, :], d_wea.ap().rearrange("(k p) o -> p k o", p=128))
            gdma(t_we2[:, :], d_we2.ap().rearrange("(c p) -> p c", p=128))
            gdma(t_be1[:, :], d_be1.ap().rearrange("(c p) -> p c", p=128))
            gdma(t_be2[:, :], d_be2.ap())
            gdma(t_bnu1[:, :], d_bnu1.ap().rearrange("(c p) -> p c", p=128))
            gdma(t_bnu2[:, :], d_bnu2.ap().rearrange("(c p) -> p c", p=128))
            gdma(t_bnt1[:, :], d_bnt1.ap().rearrange("(c p) -> p c", p=128))
            gdma(t_bnt2[:, :], d_bnt2.ap())
            gdma(t_wnt1[:, :], d_wnt1.ap().rearrange("(k p) o -> p k o", p=128))
            gdma(t_wnt2[:, :], d_wnt2.ap().rearrange("(k p) t -> p k t", p=128))

            nc.gpsimd.memset(t_ones[:, :], 1.0)
            # trigger the sigmoid activation-table load as early as possible
            nc.scalar.activation(t_dummy[:, :], t_we2[0:1, 0:1], AF.Sigmoid)

            nc.vector.tensor_copy(t_we2b[:, :], t_we2[:, :])
            # bf16 copy of x for the hj matmuls (ACT: idle in this window)
            nc.scalar.activation(t_xTb[:, :], t_xT[:, :], AF.Identity)

            # agg = sum_n x (per b, per d-chunk), scaled to mean
            for q in range(4):
                xq = t_xT[:, q * R: (q + 1) * R].rearrange("p (b j) -> p b j", b=B)
                nc.vector.tensor_reduce(
                    t_aggs[:, q * 2: (q + 1) * 2], xq,
                    mybir.AxisListType.X, AluOpType.add)
            nc.vector.tensor_scalar(t_aggb[:, :], t_aggs[:, :], 1.0 / N, None,
                                    AluOpType.mult)

            with (
                tc.tile_pool(name="ps_mlp", bufs=2, space="PSUM") as pmlp,
                tc.tile_pool(name="ps_hjx", bufs=2, space="PSUM") as phjx,
                tc.tile_pool(name="ps_misc", bufs=1, space="PSUM") as pmisc,
            ):
                # ---- node-update MLP on agg (tiny: free dim = 2) ----
                for oq in range(4):
                    ps = pmlp.tile([128, 2], F32, tag="mlp")
                    for kq in range(4):
                        nc.tensor.matmul(
                            ps[:, :],
                            t_wnu1[:, kq * D + oq * 128: kq * D + (oq + 1) * 128],
                            t_aggb[:, kq * 2: (kq + 1) * 2],
                            start=(kq == 0), stop=(kq == 3))
                    nc.vector.tensor_scalar(
                        t_h1b[:, oq * 2: (oq + 1) * 2], ps[:, :],
                        t_bnu1[:, oq: oq + 1], 0.0,
                        AluOpType.add, AluOpType.max)
                for oq in range(4):
                    ps = pmlp.tile([128, 2], F32, tag="mlp")
                    for kq in range(4):
                        nc.tensor.matmul(
                            ps[:, :],
                            t_wnu2[:, kq * D + oq * 128: kq * D + (oq + 1) * 128],
                            t_h1b[:, kq * 2: (kq + 1) * 2],
                            start=(kq == 0), stop=(kq == 3))
                    nc.vector.tensor_scalar(
                        t_upd[:, oq * 2: (oq + 1) * 2], ps[:, :],
                        t_bnu2[:, oq: oq + 1], None, AluOpType.add)
                nc.vector.tensor_copy(t_updb[:, :], t_upd[:, :])

                # ---- biasE = upd @ web (per-partition bias cols for hj) ----
                for oq in range(4):
                    ps = pmlp.tile([128, 2], F32, tag="mlp")
                    for kq in range(4):
                        nc.tensor.matmul(
                            ps[:, :],
                            t_web[:, kq * D + oq * 128: kq * D + (oq + 1) * 128],
                            t_updb[:, kq * 2: (kq + 1) * 2],
                            start=(kq == 0), stop=(kq == 3))
                    nc.vector.tensor_copy(t_biasE[:, oq * 2: (oq + 1) * 2], ps[:, :])

                # ---- usT = xsT + upd[b] (slab u for hi / node head) ----
                for q in range(4):
                    for b in range(B):
                        sc = t_upd[:, q * 2 + b: q * 2 + b + 1]
                        dsts = t_usT[:, q * RS + b * ISLAB: q * RS + (b + 1) * ISLAB]
                        srcs = t_xsT[:, q * RS + b * ISLAB: q * RS + (b + 1) * ISLAB]
                        nc.vector.tensor_scalar(dsts, srcs, sc, None, AluOpType.add)
                nc.scalar.activation(t_usTb[:, :], t_usT[:, :], AF.Identity)

                # ---- be2 broadcast across partitions (for edge sigmoid bias)
                psb = pmisc.tile([128, 1], F32, tag="bb")
                nc.tensor.matmul(psb[:, :], t_ones[0:1, 0:128], t_be2[0:1, 0:1],
                                 start=True, stop=True)
                nc.vector.tensor_copy(t_b2col[:, :], psb[:, :])

                # ---- hjT = (x@web)T + biasE  (bf16, full j range) ----
                for oq in range(4):
                    psj = phjx.tile([128, R], F32, tag="hjx")
                    for kq in range(4):
                        nc.tensor.matmul(
                            psj[:, :],
                            t_web[:, kq * D + oq * 128: kq * D + (oq + 1) * 128],
                            t_xTb[:, kq * R: (kq + 1) * R],
                            start=(kq == 0), stop=(kq == 3))
                    for b in range(B):
                        nc.vector.tensor_scalar(
                            t_hjb[:, oq * R + b * N: oq * R + (b + 1) * N],
                            psj[:, b * N: (b + 1) * N],
                            t_biasE[:, oq * 2 + b: oq * 2 + b + 1],
                            None, AluOpType.add)

                # ---- hiT slab = (us@wea)T + be1  (fp32) ----
                psh = pmisc.tile([128, 4 * RS], F32, tag="hi")
                for oq in range(4):
                    for kq in range(4):
                        nc.tensor.matmul(
                            psh[:, oq * RS: (oq + 1) * RS],
                            t_wea[:, kq * D + oq * 128: kq * D + (oq + 1) * 128],
                            t_usTb[:, kq * RS: (kq + 1) * RS],
                            start=(kq == 0), stop=(kq == 3))
                for oq in range(4):
                    nc.vector.tensor_scalar(
                        t_hib[:, oq * RS: (oq + 1) * RS],
                        psh[:, oq * RS: (oq + 1) * RS],
                        t_be1[:, oq: oq + 1], None, AluOpType.add)

            # ---- node head (completes before the edge phase needs PSUM) ----
            with tc.tile_pool(name="ps_node", bufs=1, space="PSUM") as pnode:
                psn = pnode.tile([128, 4 * RS], F32, tag="n1")
                for oq in range(4):
                    for kq in range(4):
                        nc.tensor.matmul(
                            psn[:, oq * RS: (oq + 1) * RS],
                            t_wnt1[:, kq * D + oq * 128: kq * D + (oq + 1) * 128],
                            t_usTb[:, kq * RS: (kq + 1) * RS],
                            start=(kq == 0), stop=(kq == 3))
                for oq in range(4):
                    nc.vector.tensor_scalar(
                        t_h1n[:, oq * RS: (oq + 1) * RS],
                        psn[:, oq * RS: (oq + 1) * RS],
                        t_bnt1[:, oq: oq + 1], 0.0,
                        AluOpType.add, AluOpType.max)
                psn2 = pnode.tile([RS, 3], F32, tag="n2")
                for kq in range(4):
                    nc.tensor.matmul(
                        psn2[:, :],
                        t_h1n[:, kq * RS: (kq + 1) * RS],
                        t_wnt2[:, kq * 3: (kq + 1) * 3],
                        start=(kq == 0), stop=False)
                nc.tensor.matmul(psn2[:, :], t_ones[0:1, 0:RS], t_bnt2[0:1, :],
                                 start=False, stop=True)
                nc.scalar.activation(t_nout[:, :], psn2[:, :], AF.Sigmoid)
                nc.sync.dma_start(d_node.ap()[:, :], t_nout[:, :])

            # ---- edge main loop ----
            with (
                tc.tile_pool(name="ps_edge", bufs=1, space="PSUM") as pedge,
                tc.tile_pool(name="rpool", bufs=16) as rp,
                tc.tile_pool(name="stage", bufs=2) as stp,
            ):
                half_ctr = 0
                for b in range(B):
                    pe_t = pedge.tile([128, 4 * D], F32, tag=f"pe{b}")
                    for oq in range(4):
                        for s in range(4):
                            for m in range(4):
                                rt = rp.tile([128, D], BF16, tag="rt")
                                for h in range(2):
                                    i = 8 * s + 2 * m + h
                                    dst = rt[:, h * N: (h + 1) * N]
                                    src = t_hjb[:, oq * R + b * N: oq * R + (b + 1) * N]
                                    sc = t_hib[:, oq * RS + b * ISLAB + i:
                                               oq * RS + b * ISLAB + i + 1]
                                    e = HALF_PATTERN[half_ctr % len(HALF_PATTERN)]
                                    half_ctr += 1
                                    if e == "A":
                                        nc.scalar.activation(dst, src, AF.Relu, bias=sc)
                                    elif e == "G":
                                        nc.gpsimd.tensor_scalar(
                                            dst, src, sc, 0.0,
                                            AluOpType.add, AluOpType.max)
                                    else:
                                        nc.vector.tensor_scalar(
                                            dst, src, sc, 0.0,
                                            AluOpType.add, AluOpType.max)
                                nc.tensor.matmul(
                                    pe_t[32 * m: 32 * m + 1, s * D: (s + 1) * D],
                                    t_we2b[:, oq: oq + 1],
                                    rt[:, :],
                                    start=(oq == 0), stop=(oq == 3),
                                    tile_position=(0, 32 * m))
                    stage = stp.tile([128, 4 * D], F32, tag="stage")
                    nc.scalar.activation(stage[:, :], pe_t[:, :], AF.Sigmoid,
                                         bias=t_b2col[:, :])
                    eb = d_edge.ap()[b * ISLAB: (b + 1) * ISLAB, :].rearrange(
                        "(s m h) j -> m s h j", s=4, m=4, h=2)
                    for m in range(4):
                        nc.sync.dma_start(eb[m], stage[32 * m: 32 * m + 1, :])

    nc.finalize()
    return nc


def _get_nc():
    if "nc" not in _CACHE:
        _CACHE["nc"] = _build()
    return _CACHE["nc"]


def _prepare_in_maps(inputs):
    bf = ml_dtypes.bfloat16
    x = np.asarray(inputs["node_features"], dtype=np.float32)
    xT = np.ascontiguousarray(x.reshape(B * N, D).T)          # [D, R]
    et_w1 = np.asarray(inputs["et_w1"], np.float32)
    base = {
        "xT": xT,
        "wnu1": np.asarray(inputs["nu_w1"], np.float32).astype(bf),
        "bnu1": np.asarray(inputs["nu_b1"], np.float32),
        "wnu2": np.asarray(inputs["nu_w2"], np.float32).astype(bf),
        "bnu2": np.asarray(inputs["nu_b2"], np.float32),
        "wnt1": np.asarray(inputs["nt_w1"], np.float32).astype(bf),
        "bnt1": np.asarray(inputs["nt_b1"], np.float32),
        "wnt2": np.asarray(inputs["nt_w2"], np.float32).astype(bf),
        "bnt2": np.asarray(inputs["nt_b2"], np.float32).reshape(1, 3).astype(bf),
        "wea": np.ascontiguousarray(et_w1[:D]).astype(bf),
        "web": np.ascontiguousarray(et_w1[D:]).astype(bf),
        "be1": np.asarray(inputs["et_b1"], np.float32),
        "we2": np.ascontiguousarray(np.asarray(inputs["et_w2"], np.float32)[:, 0]),
        "be2": np.asarray(inputs["et_b2"], np.float32).reshape(1, 1).astype(bf),
    }
    in_maps = []
    for c in range(NCORES):
        xs = x[:, c * ISLAB: (c + 1) * ISLAB, :]              # [B, 32, D]
        xsT = np.ascontiguousarray(xs.reshape(RS, D).T)       # [D, RS]
        in_maps.append({**base, "xsT": xsT})
    return in_maps


def _gather(results):
    node = np.zeros((B, N, 3), np.float32)
    edge = np.zeros((B, N, N), np.float32)
    for c in range(NCORES):
        node[:, c * ISLAB: (c + 1) * ISLAB, :] = \
            results[c]["node_o"].reshape(B, ISLAB, 3)
        edge[:, c * ISLAB: (c + 1) * ISLAB, :] = \
            results[c]["edge_o"].reshape(B, ISLAB, N)
    return node.reshape(B, N * 3), edge.reshape(B, N * N)


def _run(inputs, trace=False, **kwargs):
    nc = _get_nc()
    in_maps = _prepare_in_maps(inputs)
    res = run_bass_kernel_spmd(nc, in_maps, list(range(NCORES)),
                               trace=trace, **kwargs)
    return _gather(res.results), res


def kernel(**inputs):
    (node, edge), _ = _run(inputs, trace=False)
    return node, edge


def kernel_traced(**inputs):
    """Returns ((node, edge), BassKernelResults-with-profile)."""
    return _run(inputs, trace=True)


# revision 19
# speedup vs baseline: 1.1619x; 1.0259x over previous
"""Trainium2 Bass kernel for the MessagePassingGNN problem.

Full inputs in, full outputs out. Internally shards the pairwise edge
computation along the i axis across 8 NeuronCores (each core computes
edge rows for its 32-i slab plus the node head for the same slab).

Structure (per core), with u = x + upd never materialized in full:
  aggT = mean_n x                          (DVE free-dim reduce)
  upd  = MLP(agg)                          (tiny bf16 matmuls)
  hjT  = (x @ et_w1[D:])T + (upd @ et_w1[D:])  (bf16; upd term folded in
                                            as a per-partition bias at
                                            PSUM-evacuation time)
  us   = x_slab + upd;  hi = (us @ et_w1[:D])T + et_b1
  node head on the slab
  edge: for each (i, d-chunk): R = relu(hjT + hi[:,i]) [128, 512] bf16
        (fused add+max on DVE/ACT), then a PE matmul with the w2 chunk
        [128,1] stationary reduces over d into one PSUM row per i-pair;
        sigmoid + DMA out.
The [B,N,N,D] hidden tensor of the reference is never materialized.

All inputs arrive pre-arranged by the host into the exact SBUF layouts
(partition-major [128, X]) so every DMA is a contiguous 2-D copy.
"""

import numpy as np
import ml_dtypes

import concourse.bass as bass
import concourse.bacc as bacc
import concourse.mybir as mybir
import concourse.tile as tile
from concourse.alu_op_type import AluOpType
from concourse.bass_utils import run_bass_kernel_spmd

F32 = mybir.dt.float32
BF16 = mybir.dt.bfloat16
AF = mybir.ActivationFunctionType

B, N, D = 2, 256, 512
NCORES = 8
ISLAB = N // NCORES          # 32 i-rows per core
R = B * N                    # 512 flattened rows (b, n)
RS = B * ISLAB               # 64 flattened slab rows (b, ii)

# relu half-op engine pattern: V = vector/DVE, A = scalar/ACT.
# (gpsimd is ~14x slower for this op and steals DVE's SBUF port.)
HALF_PATTERN = "VVA"

_CACHE = {}


def _build():
    nc = bacc.Bacc("TRN2", target_bir_lowering=False, debug=False,
                   num_devices=NCORES)

    # -------- DRAM I/O (host pre-arranged to SBUF layouts) --------
    d_xTb = [nc.dram_tensor(f"xTb{q}", [128, R], BF16, kind="ExternalInput")
             for q in range(4)]
    d_xsTb = nc.dram_tensor("xsTb", [128, 4 * RS], BF16, kind="ExternalInput")
    d_wnu1 = nc.dram_tensor("wnu1", [128, 4 * D], BF16, kind="ExternalInput")
    d_wnu2 = nc.dram_tensor("wnu2", [128, 4 * D], BF16, kind="ExternalInput")
    d_wnt1 = nc.dram_tensor("wnt1", [128, 4 * D], BF16, kind="ExternalInput")
    d_wnt2 = nc.dram_tensor("wnt2", [128, 12], BF16, kind="ExternalInput")
    d_wea = nc.dram_tensor("wea", [128, 4 * D], BF16, kind="ExternalInput")
    d_web = nc.dram_tensor("web", [128, 4 * D], BF16, kind="ExternalInput")
    d_bnu1 = nc.dram_tensor("bnu1", [128, 4], F32, kind="ExternalInput")
    d_bnu2 = nc.dram_tensor("bnu2", [128, 4], F32, kind="ExternalInput")
    d_bnt1 = nc.dram_tensor("bnt1", [128, 4], F32, kind="ExternalInput")
    d_bnt2 = nc.dram_tensor("bnt2", [1, 3], BF16, kind="ExternalInput")
    d_be1 = nc.dram_tensor("be1", [128, 4], F32, kind="ExternalInput")
    d_we2 = nc.dram_tensor("we2", [128, 4], BF16, kind="ExternalInput")
    d_be2 = nc.dram_tensor("be2", [1, 1], BF16, kind="ExternalInput")

    d_node = nc.dram_tensor("node_o", [RS, 3], F32, kind="ExternalOutput")
    d_edge = nc.dram_tensor("edge_o", [RS, N], F32, kind="ExternalOutput")

    with tile.TileContext(nc) as tc:
        with tc.tile_pool(name="const", bufs=1) as cp:
            # ---------------- SBUF residents ----------------
            t_xTb = cp.tile([128, 4 * R], BF16, tag="xTb")     # [p, q*512+r]
            t_xsTb = cp.tile([128, 4 * RS], BF16, tag="xsTb")  # [p, q*64+rs]
            t_usTb = cp.tile([128, 4 * RS], BF16, tag="usTb")
            t_wnu1 = cp.tile([128, 4 * D], BF16, tag="wnu1")   # [p, kq*512+o]
            t_wnu2 = cp.tile([128, 4 * D], BF16, tag="wnu2")
            t_wnt1 = cp.tile([128, 4 * D], BF16, tag="wnt1")
            t_wea = cp.tile([128, 4 * D], BF16, tag="wea")
            t_web = cp.tile([128, 4 * D], BF16, tag="web")
            t_wnt2 = cp.tile([128, 12], BF16, tag="wnt2")      # [p, kq*3+t]
            t_bnu1 = cp.tile([128, 4], F32, tag="bnu1")        # [p, chunk]
            t_bnu2 = cp.tile([128, 4], F32, tag="bnu2")
            t_bnt1 = cp.tile([128, 4], F32, tag="bnt1")
            t_be1 = cp.tile([128, 4], F32, tag="be1")
            t_we2b = cp.tile([128, 4], BF16, tag="we2b")
            t_bnt2 = cp.tile([1, 3], BF16, tag="bnt2")
            t_be2 = cp.tile([1, 1], BF16, tag="be2")
            t_ones = cp.tile([1, 128], BF16, tag="ones")
            t_dummy = cp.tile([1, 1], F32, tag="dummy")

            t_aggs = cp.tile([128, 8], F32, tag="aggs")        # [p, q*2+b]
            t_aggb = cp.tile([128, 8], BF16, tag="aggb")
            t_h1b = cp.tile([128, 8], BF16, tag="h1b")
            t_upd = cp.tile([128, 8], F32, tag="upd")
            t_updb = cp.tile([128, 8], BF16, tag="updb")
            t_biasE = cp.tile([128, 8], F32, tag="biasE")      # upd@web cols
            t_b2col = cp.tile([128, 1], F32, tag="b2col")      # be2 bcast

            t_hjb = cp.tile([128, 4 * R], BF16, tag="hjb")     # [p, q*512+(b*256+j)]
            t_hib = cp.tile([128, 4 * RS], F32, tag="hib")     # [p, q*64+(b*32+ii)]
            t_h1n = cp.tile([128, 4 * RS], BF16, tag="h1n")
            t_nout = cp.tile([RS, 3], F32, tag="nout")

            # ------- input DMAs: critical path on the sync HWDGE ring,
            # the rest on the gpsimd SWDGE ring -------
            sdma = nc.sync.dma_start
            gdma = nc.gpsimd.dma_start
            for q in range(4):
                sdma(t_xTb[:, q * R: (q + 1) * R], d_xTb[q].ap())
            sdma(t_web[:, :], d_web.ap())
            sdma(t_wnu1[:, :], d_wnu1.ap())
            sdma(t_wnu2[:, :], d_wnu2.ap())
            gdma(t_xsTb[:, :], d_xsTb.ap())
            gdma(t_wea[:, :], d_wea.ap())
            gdma(t_we2b[:, :], d_we2.ap())
            gdma(t_be1[:, :], d_be1.ap())
            gdma(t_be2[:, :], d_be2.ap())
            gdma(t_bnu1[:, :], d_bnu1.ap())
            gdma(t_bnu2[:, :], d_bnu2.ap())
            gdma(t_bnt1[:, :], d_bnt1.ap())
            gdma(t_bnt2[:, :], d_bnt2.ap())
            gdma(t_wnt1[:, :], d_wnt1.ap())
            gdma(t_wnt2[:, :], d_wnt2.ap())

            nc.gpsimd.memset(t_ones[:, :], 1.0)
            # trigger the sigmoid activation-table load as early as possible
            nc.scalar.activation(t_dummy[:, :], t_dummy[:, :], AF.Sigmoid)

            # agg = mean_n x (per b, per d-chunk)
            for q in range(4):
                xq = t_xTb[:, q * R: (q + 1) * R].rearrange("p (b j) -> p b j", b=B)
                nc.vector.tensor_reduce(
                    t_aggs[:, q * 2: (q + 1) * 2], xq,
                    mybir.AxisListType.X, AluOpType.add)
            nc.vector.tensor_scalar(t_aggb[:, :], t_aggs[:, :], 1.0 / N, None,
                                    AluOpType.mult)

            with (
                tc.tile_pool(name="ps_mlp", bufs=2, space="PSUM") as pmlp,
                tc.tile_pool(name="ps_hjx", bufs=2, space="PSUM") as phjx,
                tc.tile_pool(name="ps_misc", bufs=1, space="PSUM") as pmisc,
            ):
                # ---- node-update MLP on agg (tiny: free dim = 2) ----
                for oq in range(4):
                    ps = pmlp.tile([128, 2], F32, tag="mlp")
                    for kq in range(4):
                        nc.tensor.matmul(
                            ps[:, :],
                            t_wnu1[:, kq * D + oq * 128: kq * D + (oq + 1) * 128],
                            t_aggb[:, kq * 2: (kq + 1) * 2],
                            start=(kq == 0), stop=(kq == 3))
                    nc.vector.tensor_scalar(
                        t_h1b[:, oq * 2: (oq + 1) * 2], ps[:, :],
                        t_bnu1[:, oq: oq + 1], 0.0,
                        AluOpType.add, AluOpType.max)
                for oq in range(4):
                    ps = pmlp.tile([128, 2], F32, tag="mlp")
                    for kq in range(4):
                        nc.tensor.matmul(
                            ps[:, :],
                            t_wnu2[:, kq * D + oq * 128: kq * D + (oq + 1) * 128],
                            t_h1b[:, kq * 2: (kq + 1) * 2],
                            start=(kq == 0), stop=(kq == 3))
                    nc.vector.tensor_scalar(
                        t_upd[:, oq * 2: (oq + 1) * 2], ps[:, :],
                        t_bnu2[:, oq: oq + 1], None, AluOpType.add)
                nc.vector.tensor_copy(t_updb[:, :], t_upd[:, :])

                # ---- biasE = upd @ web (per-partition bias cols for hj) ----
                for oq in range(4):
                    ps = pmlp.tile([128, 2], F32, tag="mlp")
                    for kq in range(4):
                        nc.tensor.matmul(
                            ps[:, :],
                            t_web[:, kq * D + oq * 128: kq * D + (oq + 1) * 128],
                            t_updb[:, kq * 2: (kq + 1) * 2],
                            start=(kq == 0), stop=(kq == 3))
                    nc.vector.tensor_copy(t_biasE[:, oq * 2: (oq + 1) * 2], ps[:, :])

                # ---- usT = xsT + upd[b] (slab u for hi / node head) ----
                for q in range(4):
                    for b in range(B):
                        sc = t_upd[:, q * 2 + b: q * 2 + b + 1]
                        dsts = t_usTb[:, q * RS + b * ISLAB: q * RS + (b + 1) * ISLAB]
                        srcs = t_xsTb[:, q * RS + b * ISLAB: q * RS + (b + 1) * ISLAB]
                        nc.vector.tensor_scalar(dsts, srcs, sc, None, AluOpType.add)

                # ---- be2 broadcast across partitions (edge sigmoid bias) ----
                psb = pmisc.tile([128, 1], F32, tag="bb")
                nc.tensor.matmul(psb[:, :], t_ones[0:1, 0:128], t_be2[0:1, 0:1],
                                 start=True, stop=True)
                nc.vector.tensor_copy(t_b2col[:, :], psb[:, :])

                # ---- hjT = (x@web)T + biasE  (bf16, full j range) ----
                for oq in range(4):
                    psj = phjx.tile([128, R], F32, tag="hjx")
                    for kq in range(4):
                        nc.tensor.matmul(
                            psj[:, :],
                            t_web[:, kq * D + oq * 128: kq * D + (oq + 1) * 128],
                            t_xTb[:, kq * R: (kq + 1) * R],
                            start=(kq == 0), stop=(kq == 3))
                    for b in range(B):
                        nc.vector.tensor_scalar(
                            t_hjb[:, oq * R + b * N: oq * R + (b + 1) * N],
                            psj[:, b * N: (b + 1) * N],
                            t_biasE[:, oq * 2 + b: oq * 2 + b + 1],
                            None, AluOpType.add)

                # ---- hiT slab = (us@wea)T + be1  (fp32 out) ----
                psh = pmisc.tile([128, 4 * RS], F32, tag="hi")
                for oq in range(4):
                    for kq in range(4):
                        nc.tensor.matmul(
                            psh[:, oq * RS: (oq + 1) * RS],
                            t_wea[:, kq * D + oq * 128: kq * D + (oq + 1) * 128],
                            t_usTb[:, kq * RS: (kq + 1) * RS],
                            start=(kq == 0), stop=(kq == 3))
                for oq in range(4):
                    nc.vector.tensor_scalar(
                        t_hib[:, oq * RS: (oq + 1) * RS],
                        psh[:, oq * RS: (oq + 1) * RS],
                        t_be1[:, oq: oq + 1], None, AluOpType.add)

            # ---- node head (completes before the edge phase needs PSUM) ----
            with tc.tile_pool(name="ps_node", bufs=1, space="PSUM") as pnode:
                psn = pnode.tile([128, 4 * RS], F32, tag="n1")
                for oq in range(4):
                    for kq in range(4):
                        nc.tensor.matmul(
                            psn[:, oq * RS: (oq + 1) * RS],
                            t_wnt1[:, kq * D + oq * 128: kq * D + (oq + 1) * 128],
                            t_usTb[:, kq * RS: (kq + 1) * RS],
                            start=(kq == 0), stop=(kq == 3))
                for oq in range(4):
                    nc.vector.tensor_scalar(
                        t_h1n[:, oq * RS: (oq + 1) * RS],
                        psn[:, oq * RS: (oq + 1) * RS],
                        t_bnt1[:, oq: oq + 1], 0.0,
                        AluOpType.add, AluOpType.max)
                psn2 = pnode.tile([RS, 3], F32, tag="n2")
                for kq in range(4):
                    nc.tensor.matmul(
                        psn2[:, :],
                        t_h1n[:, kq * RS: (kq + 1) * RS],
                        t_wnt2[:, kq * 3: (kq + 1) * 3],
                        start=(kq == 0), stop=False)
                nc.tensor.matmul(psn2[:, :], t_ones[0:1, 0:RS], t_bnt2[0:1, :],
                                 start=False, stop=True)
                nc.scalar.activation(t_nout[:, :], psn2[:, :], AF.Sigmoid)
                nc.sync.dma_start(d_node.ap()[:, :], t_nout[:, :])

            # ---- edge main loop ----
            with (
                tc.tile_pool(name="ps_edge", bufs=1, space="PSUM") as pedge,
                tc.tile_pool(name="rpool", bufs=16) as rp,
                tc.tile_pool(name="stage", bufs=2) as stp,
            ):
                half_ctr = 0
                for b in range(B):
                    pe_t = pedge.tile([128, 4 * D], F32, tag=f"pe{b}")
                    for oq in range(4):
                        for s in range(4):
                            for m in range(4):
                                rt = rp.tile([128, D], BF16, tag="rt")
                                for h in range(2):
                                    i = 8 * s + 2 * m + h
                                    dst = rt[:, h * N: (h + 1) * N]
                                    src = t_hjb[:, oq * R + b * N: oq * R + (b + 1) * N]
                                    sc = t_hib[:, oq * RS + b * ISLAB + i:
                                               oq * RS + b * ISLAB + i + 1]
                                    e = HALF_PATTERN[half_ctr % len(HALF_PATTERN)]
                                    half_ctr += 1
                                    if e == "A":
                                        nc.scalar.activation(dst, src, AF.Relu, bias=sc)
                                    else:
                                        nc.vector.tensor_scalar(
                                            dst, src, sc, 0.0,
                                            AluOpType.add, AluOpType.max)
                                nc.tensor.matmul(
                                    pe_t[32 * m: 32 * m + 1, s * D: (s + 1) * D],
                                    t_we2b[:, oq: oq + 1],
                                    rt[:, :],
                                    start=(oq == 0), stop=(oq == 3),
                                    tile_position=(0, 32 * m))
                    stage = stp.tile([128, 4 * D], F32, tag="stage")
                    nc.scalar.activation(stage[:, :], pe_t[:, :], AF.Sigmoid,
                                         bias=t_b2col[:, :])
                    eb = d_edge.ap()[b * ISLAB: (b + 1) * ISLAB, :].rearrange(
                        "(s m h) j -> m s h j", s=4, m=4, h=2)
                    for m in range(4):
                        nc.sync.dma_start(eb[m], stage[32 * m: 32 * m + 1, :])

    nc.finalize()
    return nc


def _get_nc():
    if "nc" not in _CACHE:
        _CACHE["nc"] = _build()
    return _CACHE["nc"]


def _arr_w(w):
    """[512, O] fp32 -> [128, 4*O] bf16 in k-chunk-major SBUF layout."""
    w = np.asarray(w, np.float32)
    o = w.shape[1]
    return np.ascontiguousarray(
        w.reshape(4, 128, o).transpose(1, 0, 2).reshape(128, 4 * o)
    ).astype(ml_dtypes.bfloat16)


def _arr_b(v):
    """[512] fp32 -> [128, 4] chunk-column layout (fp32)."""
    v = np.asarray(v, np.float32)
    return np.ascontiguousarray(v.reshape(4, 128).T)


def _prepare_in_maps(inputs):
    bf = ml_dtypes.bfloat16
    x = np.asarray(inputs["node_features"], dtype=np.float32)
    xT = x.reshape(B * N, D).T                                # [D, R] view
    et_w1 = np.asarray(inputs["et_w1"], np.float32)
    base = {
        "wnu1": _arr_w(inputs["nu_w1"]),
        "bnu1": _arr_b(inputs["nu_b1"]),
        "wnu2": _arr_w(inputs["nu_w2"]),
        "bnu2": _arr_b(inputs["nu_b2"]),
        "wnt1": _arr_w(inputs["nt_w1"]),
        "bnt1": _arr_b(inputs["nt_b1"]),
        "wnt2": _arr_w(inputs["nt_w2"]),
        "bnt2": np.asarray(inputs["nt_b2"], np.float32).reshape(1, 3).astype(bf),
        "wea": _arr_w(et_w1[:D]),
        "web": _arr_w(et_w1[D:]),
        "be1": _arr_b(inputs["et_b1"]),
        "we2": _arr_b(np.asarray(inputs["et_w2"], np.float32)[:, 0]).astype(bf),
        "be2": np.asarray(inputs["et_b2"], np.float32).reshape(1, 1).astype(bf),
    }
    for q in range(4):
        base[f"xTb{q}"] = np.ascontiguousarray(
            xT[q * 128: (q + 1) * 128, :]).astype(bf)
    in_maps = []
    for c in range(NCORES):
        xs = x[:, c * ISLAB: (c + 1) * ISLAB, :]              # [B, 32, D]
        xsT = xs.reshape(RS, D).T                             # [D, RS] view
        xsTb = np.ascontiguousarray(
            xsT.reshape(4, 128, RS).transpose(1, 0, 2).reshape(128, 4 * RS)
        ).astype(bf)
        in_maps.append({**base, "xsTb": xsTb})
    return in_maps


def _gather(results):
    node = np.zeros((B, N, 3), np.float32)
    edge = np.zeros((B, N, N), np.float32)
    for c in range(NCORES):
        node[:, c * ISLAB: (c + 1) * ISLAB, :] = \
            results[c]["node_o"].reshape(B, ISLAB, 3)
        edge[:, c * ISLAB: (c + 1) * ISLAB, :] = \
            results[c]["edge_o"].reshape(B, ISLAB, N)
    return node.reshape(B, N * 3), edge.reshape(B, N * N)


def _run(inputs, trace=False, **kwargs):
    nc = _get_nc()
    in_maps = _prepare_in_maps(inputs)
    res = run_bass_kernel_spmd(nc, in_maps, list(range(NCORES)),
                               trace=trace, **kwargs)
    return _gather(res.results), res


def kernel(**inputs):
    (node, edge), _ = _run(inputs, trace=False)
    return node, edge


def kernel_traced(**inputs):
    """Returns ((node, edge), BassKernelResults-with-profile)."""
    return _run(inputs, trace=True)
